# revision 45
# baseline (speedup 1.0000x reference)
"""Trainium2 Bass kernel for nn_LocalFWL (GNN link prediction, LocalFWL-style).

Strategy:
 - Host does integer-only index preprocessing: edge-count matrix (A+I, exact
   small ints, shipped as bf16), last-write-wins dedup of duplicate edges, and
   per-query path-pair enumeration (the sparse structure of the D1@D2
   contraction) expressed as gather indices plus a one-hot pair->query
   summation matrix. All floating-point math runs on device.
 - 8-core SPMD: the two GCN layers run as dense (A+I) bf16 matmuls on the
   tensor engine (replicated). The aggregation streams the big matrix as the
   moving operand (hT orientation) so each layer is 16 wide matmuls instead
   of 64 narrow ones; deg^-1/2 factors fold into the streamed operand and the
   layer-2 input scale, so the adjacency needs zero vector work. Queries
   (pos pairs), edge-MLPs and the pair contraction are sharded by query.
 - All irregular access runs through two transposed SBUF-source dma_gathers
   (split so the second transfer overlaps the first half's MLPs): node
   features live in SBUF as 128-wide stripes [h | 1 | 0-pad]; the gather
   emits channels-on-partitions columns, so edge feature products, the
   ones-row for the bias fold, and the final-MLP operand layouts all come
   out transpose-free. The edge MLPs run directly in pair-slot order (mlp2
   on each pair's a-edge, mlp1 on its b-edge); mean-centered MLP weights
   make the matmul emit z - mean(z) directly; the per-query pair sum is a
   one-hot matmul.
 - Inputs ship in four packed DMAs (f32 params / bf16 params+indices /
   two adjacency halves) to amortize per-DMA descriptor overhead.
"""
import sys

sys.path.insert(0, "/opt/pypackages")
sys.path.insert(0, "/opt/trn_rl_repo")

import numpy as np
import ml_dtypes

BF16 = ml_dtypes.bfloat16
FP8 = ml_dtypes.float8_e4m3

N, E, P, IN, H = 1024, 65536, 1024, 128, 64
NCORES = 8
QPC = P // NCORES
LN_EPS = 1e-5
NB = N // 128  # 8 node blocks

# packed f32 param column offsets
_PK = dict(Wm1=0, Wm2=64, Wa=128, Wb=192, bac=193, bbr=194, degpf=195,
           bg1r=203, bg2r=267, gm1r=331, gm2r=395, bem1r=459, bem2r=523,
           bm1r=587, bm2r=651)
_PKW = 715


def _ceil_to(x, m):
    return ((x + m - 1) // m) * m


def preprocess(ei, pos):
    """All-integer index preprocessing."""
    ei0 = np.asarray(ei[0], np.int64)
    ei1 = np.asarray(ei[1], np.int64)
    p0 = np.asarray(pos[0], np.int64)
    p1 = np.asarray(pos[1], np.int64)

    flat = ei0 * N + ei1
    cnt = np.bincount(flat, minlength=N * N).reshape(N, N)  # [r, c] multiplicity
    cntP = cnt + np.eye(N, dtype=np.int64)  # A + I (GCN self loop)
    deg = np.bincount(ei1, minlength=N) + 1

    last = np.full(N * N, -1, np.int64)
    last[flat] = np.arange(E)  # last occurrence wins (matches jnp .at[].set)
    PA = last.reshape(N, N)
    rowoks = PA >= 0

    per_core = []
    maxNP = 1
    for c in range(NCORES):
        qs = slice(c * QPC, (c + 1) * QPC)
        i_q, j_q = p0[qs], p1[qs]
        qid, aid, bid = [], [], []
        for q in range(QPC):
            ks = np.nonzero(rowoks[i_q[q]] & rowoks[:, j_q[q]])[0]
            if len(ks):
                qid.append(np.full(len(ks), q))
                aid.append(PA[i_q[q], ks])
                bid.append(PA[ks, j_q[q]])
        if qid:
            qid = np.concatenate(qid)
            aid = np.concatenate(aid)
            bid = np.concatenate(bid)
        else:
            qid = np.zeros(0, np.int64)
            aid = np.zeros(0, np.int64)
            bid = np.zeros(0, np.int64)
        per_core.append((i_q, j_q, qid, aid, bid))
        maxNP = max(maxNP, len(qid))

    K_J = _ceil_to(maxNP, 128) // 128
    np_pad = K_J * 128

    cores = []
    for c in range(NCORES):
        i_q, j_q, qid, aid, bid = per_core[c]
        npair = len(qid)
        # two gather halves (pipelined on separate SWDGE queues): half 0
        # covers k-blocks [0, KH), half 1 covers [KH, K_J) plus pos pairs
        KH = (K_J + 1) // 2
        lo = KH * 128

        def padded(node_ids):
            v = np.zeros(np_pad, np.int64)
            v[:npair] = node_ids
            return v

        a0 = padded(ei0[aid]); a1 = padded(ei1[aid])
        b0 = padded(ei0[bid]); b1 = padded(ei1[bid])
        g0 = np.concatenate([a0[:lo], a1[:lo], b0[:lo], b1[:lo]]).astype(np.int16)
        g1 = np.concatenate([a0[lo:], a1[lo:], b0[lo:], b1[lo:],
                             i_q, j_q]).astype(np.int16)

        def wrap(g):
            n = len(g)
            iw = np.zeros((16, n // 16), np.int16)
            iw[np.arange(n) % 16, np.arange(n) // 16] = g
            return iw

        iw = np.concatenate([wrap(g0), wrap(g1)], axis=1)
        Sp = np.zeros((128, np_pad), np.float32)
        if npair:
            s = np.arange(npair)
            Sp[s % 128, (s // 128) * 128 + qid[:npair]] = 1.0
        cores.append(dict(
            gidx16=np.ascontiguousarray(np.tile(iw, (8, 1))),
            Sp=np.ascontiguousarray(Sp.astype(BF16)),
        ))
    shared = dict(
        cntL=np.ascontiguousarray(
            cntP.astype(FP8).reshape(NB, 128, N).transpose(1, 0, 2)
            .reshape(128, NB * N)),
        deg=deg.astype(np.float32),
        K_J=K_J)
    return shared, cores


_PROGRAM_CACHE = {}


def build_program(K_J, zg1, zg2, zm):
    """zg1/zg2: b_g1/b_g2 are all-zero; zm: LN beta terms are all-zero."""
    import concourse.bacc as bacc
    import concourse.bass as bass
    import concourse.tile as tile
    import concourse.mybir as mybir
    from concourse.masks import make_identity

    dt = mybir.dt
    f32 = dt.float32
    bf = dt.bfloat16
    AF = mybir.ActivationFunctionType
    OP = mybir.AluOpType
    KP = K_J * 128
    KH = (K_J + 1) // 2
    LO, HI = KH * 128, KP - KH * 128
    NG0, NG1 = 4 * LO, 4 * HI + 2 * 128
    NG = NG0 + NG1
    # megaX bf16 pack (needed early): xT | Wg1 | Wg2
    # megaB bf16 pack (needed late): Sp | gather-idx bits
    MO_W1, MO_W2 = N, N + 64
    MWA = N + 128
    MO_GI = KP
    MWB = KP + NG // 16
    nc = bacc.Bacc("TRN2", target_bir_lowering=False, debug=False,
                   num_swdge_queues=2)

    def din(name, shape, d=f32):
        return nc.dram_tensor(name, shape, d, kind="ExternalInput").ap()

    pkD = din("pkD", [128, _PKW])
    megaX = din("megaX", [128, MWA], bf)
    megaB = din("megaB", [128, MWB], bf)
    cntL = din("cntL", [128, NB * N], dt.float8e4)
    h_tbl = nc.dram_tensor("h_tbl", [N, 128], bf).ap()
    outq = nc.dram_tensor("outq", [128, 1], f32, kind="ExternalOutput").ap()

    with tile.TileContext(nc) as tc:
        with tc.tile_pool(name="const", bufs=1) as cp, \
             tc.tile_pool(name="work", bufs=3) as wp, \
             tc.tile_pool(name="psum", bufs=4, space="PSUM") as pp, \
             tc.tile_pool(name="psumB", bufs=1, space="PSUM") as ppB:

            # ---- loads: 5 packed DMAs, one queue so transfer order is
            # exactly arrival-need order ----
            pkf_s = cp.tile([128, _PKW], f32)
            nc.scalar.dma_start(pkf_s[:], pkD[:])
            megaA_s = cp.tile([128, MWA], bf)
            nc.scalar.dma_start(megaA_s[:], megaX[:])
            cnt0 = cp.tile([128, NB * N // 2], dt.float8e4)
            nc.scalar.dma_start(cnt0[:], cntL[:, 0:NB * N // 2])
            cnt1 = cp.tile([128, NB * N // 2], dt.float8e4)
            nc.scalar.dma_start(cnt1[:], cntL[:, NB * N // 2:])
            megaB_s = cp.tile([128, MWB], bf)
            nc.scalar.dma_start(megaB_s[:], megaB[:])

            def cnt_rhs(rb, half):
                t = cnt0 if rb < 4 else cnt1
                o = (rb % 4) * N + half * 512
                return t[:, o:o + 512]

            xT_s = megaA_s[:, 0:N]
            Wg1_s = megaA_s[:, MO_W1:MO_W1 + 64]
            Wg2_s = megaA_s[0:H, MO_W2:MO_W2 + 64]
            S_s = megaB_s[:, 0:KP]
            gi_s = megaB_s[:, MO_GI:MWB].bitcast(dt.int16)

            def pk(name, rows, cols):
                c0 = _PK[name]
                return pkf_s[0:rows, c0:c0 + cols]


            Wm1_s = pk("Wm1", H, H); Wm2_s = pk("Wm2", H, H)
            Wa_s = pk("Wa", 2 * H, H); Wb_s = pk("Wb", H, 1)
            bac_s = pk("bac", H, 1); bbr_s = pk("bbr", 128, 1)
            degpf_s = pk("degpf", 128, NB)
            bg2_s = pk("bg2r", 128, H)
            gm1_s = pk("gm1r", 128, H); gm2_s = pk("gm2r", 128, H)
            bem1_s = pk("bem1r", 128, H); bem2_s = pk("bem2r", 128, H)
            bm1_s = pk("bm1r", 128, H); bm2_s = pk("bm2r", 128, H)

            identb = cp.tile([128, 128], bf)
            make_identity(nc, identb[:])

            # preload the sqrt activation table off the critical path
            eps_t = cp.tile([128, 1], f32)
            nc.vector.memset(eps_t[:], LN_EPS)
            warm_t = wp.tile([1, 1], f32, tag="warm")
            nc.scalar.activation(warm_t[:], eps_t[0:1, 0:1], AF.Sqrt)

            # ---- dinv = deg^-1/2 ----
            dinvpf = cp.tile([128, NB], f32)
            nc.scalar.activation(dinvpf[:], degpf_s, AF.Sqrt)
            nc.vector.reciprocal(dinvpf[:], dinvpf[:])
            dinv2pf = cp.tile([128, NB], f32)
            nc.vector.tensor_tensor(out=dinv2pf[:], in0=dinvpf[:],
                                    in1=dinvpf[:], op=OP.mult)

            # ---- centered edge-MLP weights with bias row ----
            def center_w(Ws, brow, name):
                wc = cp.tile([H + 1, H], bf, tag=name)
                wbar = wp.tile([H, 1], f32, tag="wbar")
                nc.vector.tensor_reduce(wbar[:], Ws, mybir.AxisListType.X,
                                        OP.add)
                nc.vector.tensor_scalar_mul(wbar[:], wbar[:], 1.0 / H)
                nc.vector.tensor_scalar(out=wc[0:H, :], in0=Ws,
                                        scalar1=wbar[:, 0:1], scalar2=None,
                                        op0=OP.subtract)
                bbar = wp.tile([1, 1], f32, tag="bbar")
                nc.vector.tensor_reduce(bbar[:], brow[0:1, :],
                                        mybir.AxisListType.X, OP.add)
                nc.vector.tensor_scalar_mul(bbar[:], bbar[:], 1.0 / H)
                nc.vector.tensor_scalar(out=wc[H:H + 1, :], in0=brow[0:1, :],
                                        scalar1=bbar[:, 0:1], scalar2=None,
                                        op0=OP.subtract)
                return wc

            Wc1 = center_w(Wm1_s, bm1_s, "Wc1")
            Wc2 = center_w(Wm2_s, bm2_s, "Wc2")

            # ---- node-feature stripes for the SBUF-source gather:
            # [h(64) | 1 | zeros(63)] per node block ----
            hh2 = cp.tile([128, NB * 128], bf)
            nc.vector.memset(hh2[:], 0.0)
            for cb in range(NB):
                nc.gpsimd.memset(hh2[:, cb * 128 + H:cb * 128 + H + 1], 1.0)

            # ---- GCN, hT-orientation aggregation:
            # u[h, c] = sum_r xwd[r, h] * cntP[r, c], xwd = xw * dinv[r].
            # Layer-2 input: xw2 = h1 @ W2 with h1 = u1*dinv[c] + b1 folded
            # into per-partition scales after the (u1T-slice @ W2) matmul. ----
            xwd1 = cp.tile([128, NB * H], bf)
            for b in range(NB):
                ps = pp.tile([128, H], f32, tag="ps")
                nc.tensor.matmul(ps[:], lhsT=xT_s[:, b * 128:(b + 1) * 128],
                                 rhs=Wg1_s, start=True, stop=True)
                nc.vector.tensor_scalar(out=xwd1[:, b * H:(b + 1) * H],
                                        in0=ps[:],
                                        scalar1=dinvpf[:, b:b + 1],
                                        scalar2=None, op0=OP.mult)

            def agg(xwd, tagA, tagB):
                ua = ppB.tile([H, 512], f32, tag=tagA)
                ub = ppB.tile([H, 512], f32, tag=tagB)
                for half, u in ((0, ua), (1, ub)):
                    for rb in range(NB):
                        nc.tensor.matmul(u[:],
                                         lhsT=xwd[:, rb * H:(rb + 1) * H],
                                         rhs=cnt_rhs(rb, half),
                                         start=(rb == 0), stop=(rb == NB - 1))
                return ua, ub

            u1a, u1b = agg(xwd1, "uA0", "uA1")
            u1s = cp.tile([H, N], bf)
            nc.scalar.activation(u1s[:, 0:512], u1a[:], AF.Copy)
            nc.vector.tensor_copy(out=u1s[:, 512:768], in_=u1b[:, 0:256])
            nc.scalar.activation(u1s[:, 768:1024], u1b[:, 256:512], AF.Copy)

            # layer-2 xwd2[c,:] = dinv2[c]*(u1T[c] @ W2) [+ dinv[c]*(b1@W2)]
            xwd2 = cp.tile([128, NB * H], bf)
            bwrep = None
            if not zg1:
                # bwrep [128, H] = (b_g1 @ W_g2) replicated across partitions
                bg1b = cp.tile([1, H], bf)
                nc.vector.tensor_copy(out=bg1b[:], in_=pk("bg1r", 1, H))
                ptb = pp.tile([H, 1], bf, tag="ps")
                nc.tensor.transpose(ptb[:], bg1b[:], identb[:])
                bg1c = cp.tile([H, 1], bf)
                nc.vector.tensor_copy(out=bg1c[:], in_=ptb[:])
                psw = pp.tile([1, H], f32, tag="ps")
                nc.tensor.matmul(psw[:], lhsT=bg1c[:], rhs=Wg2_s,
                                 start=True, stop=True)
                bwb = cp.tile([1, H], bf)
                nc.vector.tensor_copy(out=bwb[:], in_=psw[:])
                ones1 = cp.tile([1, 128], bf)
                nc.vector.memset(ones1[:], 1.0)
                psr = pp.tile([128, H], f32, tag="ps")
                nc.tensor.matmul(psr[:], lhsT=ones1[:], rhs=bwb[:],
                                 start=True, stop=True)
                bwrep = cp.tile([128, H], f32)
                nc.vector.tensor_copy(out=bwrep[:], in_=psr[:])
            for cb in range(NB):
                ps = pp.tile([128, H], f32, tag="ps")
                nc.tensor.matmul(ps[:], lhsT=u1s[:, cb * 128:(cb + 1) * 128],
                                 rhs=Wg2_s, start=True, stop=True)
                if zg1:
                    nc.vector.tensor_scalar(out=xwd2[:, cb * H:(cb + 1) * H],
                                            in0=ps[:],
                                            scalar1=dinv2pf[:, cb:cb + 1],
                                            scalar2=None, op0=OP.mult)
                else:
                    t = wp.tile([128, H], f32, tag="xw2t")
                    nc.vector.tensor_scalar(out=t[:], in0=ps[:],
                                            scalar1=dinv2pf[:, cb:cb + 1],
                                            scalar2=None, op0=OP.mult)
                    nc.vector.scalar_tensor_tensor(
                        out=xwd2[:, cb * H:(cb + 1) * H], in0=bwrep[:],
                        scalar=dinvpf[:, cb:cb + 1], in1=t[:],
                        op0=OP.mult, op1=OP.add)

            u2a, u2b = agg(xwd2, "uB0", "uB1")
            u2s = cp.tile([H, N], bf)
            nc.scalar.activation(u2s[:, 0:512], u2a[:], AF.Copy)
            nc.vector.tensor_copy(out=u2s[:, 512:768], in_=u2b[:, 0:256])
            nc.scalar.activation(u2s[:, 768:1024], u2b[:, 256:512], AF.Copy)

            # h = u2*dinv[c] + b2, transposed back into the gather stripes
            for cb in range(NB):
                pt = pp.tile([128, H], bf, tag="ps")
                nc.tensor.transpose(pt[:], u2s[:, cb * 128:(cb + 1) * 128],
                                    identb[0:H, 0:H])
                if zg2:
                    nc.vector.tensor_scalar(out=hh2[:, cb * 128:cb * 128 + H],
                                            in0=pt[:],
                                            scalar1=dinvpf[:, cb:cb + 1],
                                            scalar2=None, op0=OP.mult)
                else:
                    nc.vector.scalar_tensor_tensor(
                        out=hh2[:, cb * 128:cb * 128 + H], in0=pt[:],
                        scalar=dinvpf[:, cb:cb + 1], in1=bg2_s,
                        op0=OP.mult, op1=OP.add)

            # ---- mirror the node stripes to DRAM, then two transposed
            # DRAM-source gathers (separate SWDGE queues) for every
            # irregular access; columns carry [h | 1 | 0] ----
            h_ap = h_tbl.rearrange("(cb p) e -> p cb e", cb=NB)
            hh2v = hh2[:].rearrange("p (cb e) -> p cb e", cb=NB)
            nc.sync.dma_start(h_ap, hh2v)
            G0 = cp.tile([128, 1, NG0], bf)
            nc.gpsimd.dma_gather(G0[:], h_tbl[:], gi_s[:, 0:NG0 // 16],
                                 NG0, NG0, 128, transpose=True, queue_num=0)
            G1 = cp.tile([128, 1, NG1], bf)
            nc.gpsimd.dma_gather(G1[:], h_tbl[:], gi_s[:, NG0 // 16:],
                                 NG1, NG1, 128, transpose=True, queue_num=1)
            G0f = G0[:, 0, :]
            G1f = G1[:, 0, :]

            # edge features in pair-slot order (transposed, ones-row at H);
            # separate tiles per gather half so half-0 MLPs don't wait on
            # the second gather
            xaT0 = cp.tile([128, LO], bf)
            xbT0 = cp.tile([128, LO], bf)
            nc.vector.tensor_tensor(out=xaT0[:], in0=G0f[:, 0:LO],
                                    in1=G0f[:, LO:2 * LO], op=OP.mult)
            nc.vector.tensor_tensor(out=xbT0[:], in0=G0f[:, 2 * LO:3 * LO],
                                    in1=G0f[:, 3 * LO:4 * LO], op=OP.mult)
            if HI:
                xaT1 = cp.tile([128, HI], bf)
                xbT1 = cp.tile([128, HI], bf)
                nc.vector.tensor_tensor(out=xaT1[:], in0=G1f[:, 0:HI],
                                        in1=G1f[:, HI:2 * HI], op=OP.mult)
                nc.vector.tensor_tensor(out=xbT1[:], in0=G1f[:, 2 * HI:3 * HI],
                                        in1=G1f[:, 3 * HI:4 * HI], op=OP.mult)

            # ---- edge MLPs in pair-slot order; variance on Pool, batched
            # sqrt; relu/cast split across DVE and Pool ----
            RZ = cp.tile([128, K_J, 2 * H], bf)

            def mlp_half(srcT, Wc, vsk, k, kk, par):
                # par 0 = mlp2 on a-edges (D1 left), par 1 = mlp1 on b-edges
                psd = pp.tile([128, H], f32, tag="ps")
                nc.tensor.matmul(psd[:],
                                 lhsT=srcT[0:H + 1, kk * 128:(kk + 1) * 128],
                                 rhs=Wc[:], start=True, stop=True)
                ds = wp.tile([128, H], f32, tag=f"ds{par}")
                if par:
                    nc.scalar.activation(ds[:], psd[:], AF.Copy)
                else:
                    nc.vector.tensor_copy(out=ds[:], in_=psd[:])
                sq = wp.tile([128, H], f32, tag=f"sq{par}")
                nc.vector.scalar_tensor_tensor(
                    out=sq[:], in0=ds[:], scalar=1.0, in1=ds[:],
                    op0=OP.mult, op1=OP.mult,
                    accum_out=vsk[:, par:par + 1])
                return ds

            for k in range(K_J):
                if k < KH:
                    sa, sb, kk = xaT0, xbT0, k
                else:
                    sa, sb, kk = xaT1, xbT1, k - KH
                vsk = wp.tile([128, 2], f32, tag="vsk")
                d2 = mlp_half(sa, Wc2, vsk, k, kk, 0)
                d1 = mlp_half(sb, Wc1, vsk, k, kk, 1)
                nc.scalar.activation(vsk[:], vsk[:], AF.Sqrt,
                                     bias=eps_t[:, 0:1], scale=1.0 / H)
                nc.vector.reciprocal(vsk[:], vsk[:])
                yyk = wp.tile([128, 2 * H], f32, tag="yyk")
                for par, ds, grow, berow in ((0, d2, gm2_s, bem2_s),
                                             (1, d1, gm1_s, bem1_s)):
                    ysl = yyk[:, par * H:(par + 1) * H]
                    nc.vector.scalar_tensor_tensor(
                        out=ysl, in0=ds[:],
                        scalar=vsk[:, par:par + 1],
                        in1=grow, op0=OP.mult, op1=OP.mult)
                    eng = nc.gpsimd if par else nc.vector
                    if not zm:
                        eng.tensor_add(out=ysl, in0=ysl, in1=berow)
                    eng.tensor_scalar_max(RZ[:, k, par * H:(par + 1) * H],
                                          ysl, 0.0)

            # ---- pair products + transposed one-hot per-query sum ----
            z = cp.tile([128, K_J, H], bf)
            nc.vector.tensor_tensor(out=z[:], in0=RZ[:, :, 0:H],
                                    in1=RZ[:, :, H:2 * H], op=OP.mult)
            pvT = ppB.tile([H, 128], f32, tag="uA0")
            for k in range(K_J):
                nc.tensor.matmul(pvT[:], lhsT=z[:, k, :],
                                 rhs=S_s[:, k * 128:(k + 1) * 128],
                                 start=(k == 0), stop=(k == K_J - 1))

            # ---- featT = [pos_valT ; xxT] ----
            featT = cp.tile([128, 128], f32)
            nc.scalar.activation(featT[0:H, :], pvT[:], AF.Copy)
            gp = 4 * HI
            nc.vector.tensor_tensor(out=featT[H:2 * H, :],
                                    in0=G1f[0:H, gp:gp + 128],
                                    in1=G1f[0:H, gp + 128:gp + 256],
                                    op=OP.mult)

            # ---- final MLP, fully transposed (no PE transposes) ----
            psh = pp.tile([H, 128], f32, tag="ps")
            nc.tensor.matmul(psh[:], lhsT=Wa_s, rhs=featT[:], start=True,
                             stop=True)
            hid = cp.tile([H, 128], f32)
            nc.vector.tensor_scalar(out=hid[:], in0=psh[:],
                                    scalar1=bac_s[:, 0:1], scalar2=0.0,
                                    op0=OP.add, op1=OP.max)
            pso = pp.tile([128, 1], f32, tag="ps")
            nc.tensor.matmul(pso[:], lhsT=hid[:], rhs=Wb_s, start=True,
                             stop=True)
            ot = cp.tile([128, 1], f32)
            nc.vector.tensor_scalar(out=ot[:], in0=pso[:],
                                    scalar1=bbr_s[:, 0:1], scalar2=None,
                                    op0=OP.add)
            nc.sync.dma_start(outq[:], ot[:])

    nc.compile()
    return nc


def make_in_maps(inputs, shared, cores):
    K_J = shared["K_J"]
    KP = K_J * 128
    f = lambda k: np.ascontiguousarray(np.asarray(inputs[k], np.float32))
    pkf = np.zeros((128, _PKW), np.float32)

    def put(name, arr):
        r, c = arr.shape
        pkf[0:r, _PK[name]:_PK[name] + c] = arr

    put("Wm1", f("W_m1")); put("Wm2", f("W_m2"))
    put("Wa", f("W_a")); put("Wb", f("W_b"))
    put("bac", f("b_a").reshape(H, 1))
    put("bbr", np.broadcast_to(f("b_b").reshape(1, 1), (128, 1)))
    put("degpf", shared["deg"].reshape(NB, 128).T)
    for nm, key in [("bg1r", "b_g1"), ("bg2r", "b_g2"), ("gm1r", "g_m1"),
                    ("gm2r", "g_m2"), ("bem1r", "be_m1"), ("bem2r", "be_m2"),
                    ("bm1r", "b_m1"), ("bm2r", "b_m2")]:
        pkf[:, _PK[nm]:_PK[nm] + H] = np.broadcast_to(
            f(key).reshape(1, H), (128, H))

    megaX = np.zeros((128, N + 128), BF16)
    megaX[:, 0:N] = f("x").T.astype(BF16)
    megaX[:, N:N + 64] = f("W_g1").astype(BF16)
    megaX[0:H, N + 64:N + 128] = f("W_g2").astype(BF16)
    base = dict(cntL=shared["cntL"], megaX=np.ascontiguousarray(megaX),
                pkD=pkf)
    in_maps = []
    for c in range(NCORES):
        m = dict(base)
        gi = cores[c]["gidx16"]
        megaB = np.zeros((128, KP + gi.shape[1]), BF16)
        megaB[:, 0:KP] = cores[c]["Sp"]
        megaB[:, KP:] = gi.view(BF16)
        m["megaB"] = np.ascontiguousarray(megaB)
        in_maps.append(m)
    return in_maps


_RUNNER_CACHE = {}


def _get_runner(nc, cache_key):
    """Build (once) a jitted shard_map executor over the 8 cores, mirroring
    bass2jax.run_bass_via_pjrt but cached so repeat kernel() calls reuse the
    compiled executable instead of re-lowering."""
    if cache_key in _RUNNER_CACHE:
        return _RUNNER_CACHE[cache_key]
    import jax
    import concourse.mybir as mybir
    from concourse import bass2jax
    from jax.sharding import Mesh, PartitionSpec
    from jax.experimental.shard_map import shard_map

    bass2jax.install_neuronx_cc_hook()
    partition_name = (nc.partition_id_tensor.name
                      if nc.partition_id_tensor else None)
    in_names, out_names, out_avals, zero_shapes = [], [], [], []
    for alloc in nc.m.functions[0].allocations:
        if not isinstance(alloc, mybir.MemoryLocationSet):
            continue
        name = alloc.memorylocations[0].name
        if alloc.kind == "ExternalInput":
            if name != partition_name:
                in_names.append(name)
        elif alloc.kind == "ExternalOutput":
            out_names.append(name)
            shape = tuple(alloc.tensor_shape)
            dtype = mybir.dt.np(alloc.dtype)
            out_avals.append(jax.core.ShapedArray(shape, dtype))
            zero_shapes.append((shape, dtype))
    n_params = len(in_names)
    all_names = in_names + out_names
    if partition_name is not None:
        all_names = all_names + [partition_name]
    donate = tuple(range(n_params, n_params + len(out_names)))

    def _body(*args):
        operands = list(args)
        if partition_name is not None:
            operands.append(bass2jax.partition_id_tensor())
        outs = bass2jax._bass_exec_p.bind(
            *operands, out_avals=tuple(out_avals), in_names=tuple(all_names),
            out_names=tuple(out_names), lowering_input_output_aliases=(),
            sim_require_finite=False, sim_require_nnan=False, nc=nc)
        return tuple(outs)

    devices = jax.devices()[:NCORES]
    mesh = Mesh(np.asarray(devices), ("core",))
    n_args = n_params + len(out_names)
    sharded = jax.jit(
        shard_map(_body, mesh=mesh,
                  in_specs=(PartitionSpec("core"),) * n_args,
                  out_specs=(PartitionSpec("core"),) * len(out_names),
                  check_rep=False),
        donate_argnums=donate, keep_unused=True)
    runner = (sharded, in_names, out_names, zero_shapes)
    _RUNNER_CACHE[cache_key] = runner
    return runner


def _run_hw(nc, cache_key, in_maps):
    sharded, in_names, out_names, zero_shapes = _get_runner(nc, cache_key)
    concat_in = [np.concatenate([np.asarray(m[n]) for m in in_maps], axis=0)
                 for n in in_names]
    concat_zeros = [np.zeros((NCORES * s[0], *s[1:]), d)
                    for s, d in zero_shapes]
    out_arrs = sharded(*concat_in, *concat_zeros)
    oi = out_names.index("outq")
    return np.asarray(out_arrs[oi]).reshape(NCORES * QPC)


def kernel(**inputs):
    inputs = {k: np.asarray(v) for k, v in inputs.items()}
    shared, cores = preprocess(inputs["ei"], inputs["pos"])
    zg1 = not np.any(np.asarray(inputs["b_g1"]))
    zg2 = not np.any(np.asarray(inputs["b_g2"]))
    zm = not (np.any(np.asarray(inputs["be_m1"]))
              or np.any(np.asarray(inputs["be_m2"])))
    key = (shared["K_J"], zg1, zg2, zm)
    if key not in _PROGRAM_CACHE:
        _PROGRAM_CACHE[key] = build_program(*key)
    nc = _PROGRAM_CACHE[key]
    in_maps = make_in_maps(inputs, shared, cores)
    try:
        out = _run_hw(nc, key, in_maps)
    except Exception as e:
        # Hardware dispatch failed: execute the same compiled program per-core
        # in the simulator so the kernel still returns the program's output.
        print("hw dispatch failed (%r); falling back to CoreSim" % (e,))
        from concourse.bass_interp import CoreSim
        outs = []
        for c in range(NCORES):
            sim = CoreSim(nc, require_nnan=False, require_finite=False)
            for k, v in in_maps[c].items():
                sim.tensor(k)[:] = v
            sim.simulate(check_with_hw=False)
            outs.append(np.array(sim.tensor("outq")).reshape(QPC).copy())
        out = np.concatenate(outs)
    return out.astype(np.float32)


if __name__ == "__main__":
    import os
    os.environ.setdefault("JAX_PLATFORMS", "")
    import reference
    inputs = {k: np.asarray(v) for k, v in reference.setup_inputs().items()}
    got = kernel(**inputs)
    print(got[:8])
